# revision 1
# baseline (speedup 1.0000x reference)
"""Trainium2 Bass kernel for fused self-attention (nn_Attention).

Reference computes (only q is used; k/v inputs are dead):
    qkv = q @ in_w.T + qkv_bias ; qp,kp,vp = split(qkv)
    per head: softmax(qp @ kp.T / sqrt(hd)) @ vp
    net = concat_heads @ out_w.T + out_b

Sharding: tensor-parallel over heads. 16 heads / 8 cores = 2 heads/core.
Each core projects q against its 2-head slice of in_w, runs attention for
its (2 batch x 2 head) pairs, and computes a partial output projection
against its 128 columns of out_w. Host sums the 8 partials.

On-device layouts (matmul operands fp16, accumulation fp32 in PSUM):
  qT      [1024(d), 4096(b*2048+s)]  q transposed (host prep)
  qk_sb   [128(o), 2(Q/K), 4096(s)]  projected Q,K transposed; partition =
                                     head o-dims (h0: 0-63, h1: 64-127)
  v_sb    [128(t), b, tt, 130]       V in [token, dim] layout: h0 dims 0-63,
                                     ones col 64, h1 dims 65-128, ones col
                                     129 -> each head's PV lhsT [t, 65] slice
                                     is contiguous; the ones column makes the
                                     PV matmul also produce the softmax
                                     denominator (row 64 of pv)
  scoresT [128(t), 2(head), 512] PSUM, double-buffered; h0/h1 matmuls run
          concurrently in distinct PE row groups; one exp op per tile
  pv      [65, 512] per (head, chunk): rows 0-63 out.T, row 64 = denom
  normalize: DVE reciprocal + GpSimd partition_broadcast + DVE multiply
  proj    partial[o, s]: per (ot, s-half): 2 matmuls -> staged copy ->
          one [128, 1024] store

Scheduling: attention is an ACT(exp)-bound software pipeline (scores ->
exp one tile ahead of pv). All other work -- the rest of the b=0 QKV
projection, the entire b=1 QKV projection, and both output projections --
is split into ~1us parts and woven into specific (chunk, tt) emission
slots of the attention loops, ordered to respect streaming deadlines
(K units feed score t-tiles, V units feed pv t-tiles). Dummy matmuls
warm the PE clock gate (HAM) during the initial q-load wait, and a
dummy exp forces the ACT table load before DMAs occupy the queues.
PSUM budget: 2x2 score banks + 4 shared pv/weave banks = 8.
"""

import sys

for p in ("/opt/trn_rl_repo", "/root/.axon_site/_ro/trn_rl_repo"):
    if p not in sys.path:
        sys.path.append(p)

import numpy as np

B, S, D, H = 2, 2048, 1024, 16
BS = B * S  # 4096
HD = 64  # head dim
NCORES = 8
HPC = H // NCORES  # 2 heads per core -> 128 o-dims per core

_COMPILED = {}


def _build():
    import concourse.bass as bass  # noqa: F401
    import concourse.mybir as mybir
    import concourse.tile as tile
    from concourse import bacc
    from concourse.masks import make_identity

    f16 = mybir.dt.float16
    f32 = mybir.dt.float32
    AF = mybir.ActivationFunctionType

    nc = bacc.Bacc("TRN2", target_bir_lowering=False, debug=False,
                   num_devices=NCORES)

    qT_d = nc.declare_dram_parameter("qT", [D, BS], f16, isOutput=False)
    wqk_d = nc.declare_dram_parameter("wqk", [D, 256], f16, isOutput=False)
    wv_d = nc.declare_dram_parameter("wv", [D, 128], f16, isOutput=False)
    w2_d = nc.declare_dram_parameter("w2", [128, D], f16, isOutput=False)
    qkb_d = nc.declare_dram_parameter("qkb", [1, 256], f16, isOutput=False)
    vb_d = nc.declare_dram_parameter("vb", [1, 128], f16, isOutput=False)
    out_d = nc.declare_dram_parameter("partial", [D, BS], f16, isOutput=True)

    with tile.TileContext(nc) as tc:
        with (
            tc.tile_pool(name="persist", bufs=1) as persist,
            tc.tile_pool(name="exp", bufs=6) as exp_pool,
            tc.tile_pool(name="outT", bufs=2) as outT_pool,
            tc.tile_pool(name="recip", bufs=4) as recip_pool,
            tc.tile_pool(name="rep", bufs=4) as rep_pool,
            tc.tile_pool(name="stage", bufs=3) as stage_pool,
        ):
            # ---- resident SBUF tensors ----
            q_sb = persist.tile([128, 8, BS], f16)     # 64KB/part
            wqk_sb = persist.tile([128, 8, 256], f16)
            wv_sb = persist.tile([128, 8, 128], f16)
            w2_sb = persist.tile([128, D], f16)
            qkb_sb = persist.tile([1, 256], f16)
            vb_sb = persist.tile([1, 128], f16)
            ones_sb = persist.tile([1, 512], f16)
            qk_sb = persist.tile([128, 2, BS], f16)    # 16KB/part
            # V layout per (b, t-tile): cols 0-63 h0 dims, col 64 ones,
            # cols 65-128 h1 dims, col 129 ones -> each head's PV lhsT
            # [t, 65] slice is contiguous with its denominator in row 64
            v_sb = persist.tile([128, B, 16, 130], f16)
            ident_sb = persist.tile([128, 128], f16)
            warm_sb = persist.tile([1, 8], f32)
            nc.vector.memset(ones_sb[:, :], 1.0)
            make_identity(nc, ident_sb[:, :])
            # force the exp ACT-table load NOW, before big DMAs occupy the
            # queues -- otherwise the implicit table load lands behind them
            # and gates the first real exp by ~20us
            nc.vector.memset(warm_sb[:, :], 0.0)
            nc.scalar.activation(warm_sb[:, :], warm_sb[:, :], AF.Exp)

            # loads ordered by first use: weights for chunk-0 units first,
            # then q chunks in streaming order
            qT_t = qT_d.rearrange("(n p) m -> p n m", p=128)
            nc.sync.dma_start(wqk_sb[:, :, :],
                              wqk_d.rearrange("(n p) m -> p n m", p=128))
            nc.sync.dma_start(wv_sb[:, :, :],
                              wv_d.rearrange("(n p) m -> p n m", p=128))
            nc.sync.dma_start(qkb_sb[:, :], qkb_d[:, :])
            nc.sync.dma_start(vb_sb[:, :], vb_d[:, :])
            nc.sync.dma_start(w2_sb[:, :], w2_d[:, :])
            for scc in range(8):  # q arrives per 512-chunk: units stream
                nc.sync.dma_start(
                    q_sb[:, :, scc * 512:(scc + 1) * 512],
                    qT_t[:, :, scc * 512:(scc + 1) * 512],
                )

            # ---- work-unit emitters -------------------------------------
            # Each returns a closure that emits one psum-group of work using
            # the given pool. Units are either run solid (phase 1 for b=0) or
            # woven one-at-a-time into the attention loop's PE slack.
            def qkv_unit(pool, b, m, scc, tag, nm):
                """One projection psum-group: m=0 Q, m=1 K (-> qk_sb) or
                m=2 V (-> vT staging -> PE transpose into v_sb). Split into
                two ~1us parts so woven units never delay the exp-feeding
                score matmuls by more than ~1us on the in-order PE stream."""
                s0 = b * 2048 + scc * 512
                ref = {}

                def mm_half(lo):
                    for dk in range(lo, lo + 4):
                        w = (wqk_sb[:, dk, m * 128:(m + 1) * 128] if m < 2
                             else wv_sb[:, dk, :])
                        nc.tensor.matmul(
                            ref["ps"][:, :],
                            w,
                            q_sb[:, dk, s0:s0 + 512],
                            start=(dk == 0), stop=False,
                        )

                def part_a():
                    ref["ps"] = pool.tile([128, 512], f32, tag=tag, name=nm)
                    mm_half(0)

                def part_b():
                    ps = ref["ps"]
                    mm_half(4)
                    brow = (qkb_sb[0:1, m * 128:(m + 1) * 128] if m < 2
                            else vb_sb[0:1, :])
                    nc.tensor.matmul(  # += bias_row.T @ ones
                        ps[:, :],
                        brow,
                        ones_sb[0:1, :],
                        start=False, stop=True,
                    )
                    if m < 2:
                        nc.vector.tensor_copy(qk_sb[:, m, s0:s0 + 512], ps[:, :])
                    else:
                        vt = vt_pool.tile([128, 512], f16, tag="vt",
                                          name=f"vt{nm}")
                        nc.vector.tensor_copy(vt[:, :], ps[:, :])
                        for sub in range(4):
                            st = scc * 4 + sub
                            tr = pool.tile([128, 128], f16, tag=tag,
                                           name=f"tr{nm}_{sub}")
                            nc.tensor.transpose(
                                tr[:, :],
                                vt[:, sub * 128:(sub + 1) * 128],
                                ident_sb[:, :])
                            nc.vector.tensor_copy(v_sb[:, b, st, 0:64],
                                                  tr[:, 0:64])
                            nc.vector.tensor_copy(v_sb[:, b, st, 65:129],
                                                  tr[:, 64:128])
                            nc.vector.memset(v_sb[:, b, st, 64:65], 1.0)
                            nc.vector.memset(v_sb[:, b, st, 129:130], 1.0)
                return [part_a, part_b]

            def v_small_unit(pool, b, st, tag, nm):
                """Direct V projection for one t-tile (slower on PE but
                self-contained -> fast availability for streaming deadlines)."""
                def emit():
                    t0 = b * 2048 + st * 128
                    ps = pool.tile([128, 128], f32, tag=tag, name=nm)
                    for dk in range(8):
                        nc.tensor.matmul(
                            ps[:, :],
                            q_sb[:, dk, t0:t0 + 128],
                            wv_sb[:, dk, :],
                            start=(dk == 0), stop=False,
                        )
                    nc.tensor.matmul(
                        ps[:, :],
                        ones_sb[0:1, 0:128],
                        vb_sb[0:1, :],
                        start=False, stop=True,
                    )
                    nc.vector.tensor_copy(v_sb[:, b, st, 0:64], ps[:, 0:64])
                    nc.vector.tensor_copy(v_sb[:, b, st, 65:129],
                                          ps[:, 64:128])
                    nc.vector.memset(v_sb[:, b, st, 64:65], 1.0)
                    nc.vector.memset(v_sb[:, b, st, 129:130], 1.0)
                return [emit]

            def proj_unit(pool, b, ot, outT_sb, nm):
                def half(lo):
                    # self-contained half: 2 matmuls -> staged copy -> one
                    # [128, 1024] store; no state spans the two parts
                    stage = stage_pool.tile([128, 1024], f16, tag="st",
                                            name=f"st{nm}_{lo}")
                    for j, sc in enumerate((lo, lo + 1)):
                        ps = pool.tile([128, 512], f32, tag="pv",
                                       name=f"pj{nm}_{sc}")
                        nc.tensor.matmul(
                            ps[:, :],
                            w2_sb[:, ot * 128:(ot + 1) * 128],
                            outT_sb[:, sc, :],
                            start=True, stop=True,
                        )
                        nc.vector.tensor_copy(
                            stage[:, j * 512:(j + 1) * 512], ps[:, :])
                    nc.sync.dma_start(
                        out_d[ot * 128:(ot + 1) * 128,
                              b * 2048 + lo * 512:b * 2048 + (lo + 2) * 512],
                        stage[:, :],
                    )
                return [lambda: half(0), lambda: half(2)]

            vt_cm = tc.tile_pool(name="vt", bufs=3)
            vt_pool = vt_cm.__enter__()

            # ---- phase 1: QKV projection for b=0 chunks 0-1 (solid);
            # the rest streams into the attention loop's PE slack ----
            with tc.tile_pool(name="qkv0", bufs=4, space="PSUM") as qkv0_pool:
                # dummy matmuls fill the q-load wait: they warm the PE clock
                # gate (HAM) so the real projection runs at full rate
                wps = qkv0_pool.tile([128, 128], f32, tag="warm", name="wps")
                for i in range(80):
                    nc.tensor.matmul(wps[:, :], ident_sb[:, :], ident_sb[:, :],
                                     start=True, stop=True)
                for scc in range(2):
                    for m in range(3):
                        for part in qkv_unit(qkv0_pool, 0, m, scc, "p0",
                                             f"u0{m}{scc}"):
                            part()

            # ---- attention per b, with deferred work woven in ----
            with tc.tile_pool(name="scps", bufs=2, space="PSUM") as scps_pool, \
                 tc.tile_pool(name="pvps", bufs=4, space="PSUM") as pvps_pool:
                outT_tiles = {}
                tail_parts = []
                for b in range(B):
                    outT_sb = outT_pool.tile([128, 4, 512], f16, tag="outT",
                                             name=f"outT{b}")
                    outT_tiles[b] = outT_sb
                    # (chunk, tt) -> work units woven at that emission slot.
                    # Emission position is a hard dependency deadline: a unit
                    # feeding scores(tt)/pv(tt) must be emitted before them.
                    sched = {}

                    def assign(slots, parts):
                        assert len(slots) >= len(parts), (len(slots), len(parts))
                        for s, p in zip(slots, parts):
                            sched.setdefault(s, []).append(p)

                    if b == 0:
                        # rest of qkv(b0) ahead of its streaming deadlines
                        # (K unit scc feeds score t-tiles 4scc.., small V
                        # units feed pv t-tiles), then all of qkv(b1)
                        assign([(0, 1), (0, 2)],
                               qkv_unit(pvps_pool, 0, 1, 2, "pv", "u012"))
                        for i, st in enumerate((8, 9, 10, 11)):
                            assign([(0, 3 + i)],
                                   v_small_unit(pvps_pool, 0, st, "pv", f"vs{st}"))
                        assign([(0, 7), (0, 8)],
                               qkv_unit(pvps_pool, 0, 1, 3, "pv", "u013"))
                        for i, st in enumerate((12, 13, 14, 15)):
                            assign([(0, 9 + i)],
                                   v_small_unit(pvps_pool, 0, st, "pv", f"vs{st}"))
                        assign([(0, 13), (0, 14)],
                               qkv_unit(pvps_pool, 0, 0, 2, "pv", "u002"))
                        assign([(0, 15), (1, 1)],
                               qkv_unit(pvps_pool, 0, 0, 3, "pv", "u003"))
                        b1p = []
                        for scc in range(4):
                            for m in range(3):
                                b1p += qkv_unit(pvps_pool, 1, m, scc, "pv",
                                                f"u1{m}{scc}")
                        slots = ([(1, t) for t in range(2, 16)]
                                 + [(2, t) for t in range(1, 16, 2)]
                                 + [(3, t) for t in range(1, 16, 2)])
                        assert len(slots) >= len(b1p)
                        assign(slots, b1p)
                    else:  # projection of b=0 hides inside attention(b=1);
                        # proj(b=1) first halves ride chunks 2-3 (their outT
                        # chunks 0-1 are ready), second halves run in the tail
                        pp = []
                        for i in range(8):
                            pp += proj_unit(pvps_pool, 0, i, outT_tiles[0],
                                            f"0_{i}")
                        assign([(0, t) for t in range(1, 16, 2)]
                               + [(1, t) for t in range(1, 16, 2)], pp)
                        p1 = [proj_unit(pvps_pool, 1, i, outT_sb, f"1_{i}")
                              for i in range(8)]
                        assign([(2, t) for t in range(1, 16, 2)],
                               [u[0] for u in p1])
                        tail_parts.extend(u[1] for u in p1)
                    for ch in range(4):  # 512-wide s-chunks
                        s0 = b * 2048 + ch * 512
                        pv = [pvps_pool.tile([65, 512], f32, tag="pv",
                                             name=f"pv{b}_{ch}_{h}")
                              for h in range(HPC)]
                        prev_e = None
                        for tt in range(16):
                            t0 = b * 2048 + tt * 128
                            sc_ps = scps_pool.tile([128, 2, 512], f32, tag="sc",
                                                   name=f"sc{b}_{ch}_{tt}")
                            # h0/h1 back-to-back -> concurrent PE row groups
                            for h in range(HPC):
                                lo, hi = h * 64, (h + 1) * 64
                                nc.tensor.matmul(
                                    sc_ps[:, h, :],
                                    qk_sb[lo:hi, 1, t0:t0 + 128],
                                    qk_sb[lo:hi, 0, s0:s0 + 512],
                                    start=True, stop=True,
                                )
                            for u in sched.get((ch, tt), ()):
                                u()
                            # pv runs one iteration behind so exp(tt) overlaps
                            # pv(tt-1) and scores(tt+1) on PE
                            if prev_e is not None:
                                pe, ptt = prev_e
                                for h in range(HPC):
                                    nc.tensor.matmul(
                                        pv[h][:, :],
                                        v_sb[:, b, ptt, 65 * h:65 * h + 65],
                                        pe[:, h, :],
                                        start=(ptt == 0), stop=False,
                                    )
                            e = exp_pool.tile([128, 2, 512], f16, tag="exp",
                                              name=f"e{b}_{ch}_{tt}")
                            nc.scalar.activation(e[:, :, :], sc_ps[:, :, :],
                                                 AF.Exp, scale=0.125)
                            prev_e = (e, tt)
                        pe, ptt = prev_e
                        for h in range(HPC):
                            nc.tensor.matmul(
                                pv[h][:, :],
                                v_sb[:, b, ptt, 65 * h:65 * h + 65],
                                pe[:, h, :],
                                start=False, stop=True,
                            )
                        # normalize: denom row (64 for h0, 0 for h1) ->
                        # reciprocal -> partition broadcast -> multiply
                        for h in range(HPC):
                            recip = recip_pool.tile([1, 512], f32, tag="rc",
                                                    name=f"rc{b}{ch}{h}")
                            nc.vector.reciprocal(recip[:, :], pv[h][64:65, :])
                            rep = rep_pool.tile([64, 512], f32, tag="rep",
                                                name=f"rp{b}{ch}{h}")
                            nc.gpsimd.partition_broadcast(rep[:, :], recip[:, :])
                            nc.vector.tensor_mul(
                                outT_sb[h * 64:(h + 1) * 64, ch, :],
                                pv[h][0:64, :],
                                rep[:, :],
                            )
                for p in tail_parts:
                    p()
            vt_cm.__exit__(None, None, None)
    nc.compile()
    return nc


def _get_nc():
    if "nc" not in _COMPILED:
        _COMPILED["nc"] = _build()
    return _COMPILED["nc"]


def _prep_inputs(q, in_w, qkv_bias):
    f16 = np.float16
    qT = np.ascontiguousarray(q.transpose(2, 0, 1).reshape(D, BS)).astype(f16)
    maps = []
    for c in range(NCORES):
        r = slice(128 * c, 128 * (c + 1))
        wq, wk, wv = in_w[0:D][r], in_w[D:2 * D][r], in_w[2 * D:3 * D][r]
        maps.append({
            "qT": qT,
            "wqk": np.ascontiguousarray(np.concatenate([wq, wk], 0).T).astype(f16),
            "wv": np.ascontiguousarray(wv.T).astype(f16),
            "w2": None,  # filled with out_w slice
            "qkb": np.ascontiguousarray(
                np.concatenate([qkv_bias[0:D][r], qkv_bias[D:2 * D][r]])[None, :]
            ).astype(f16),
            "vb": np.ascontiguousarray(
                qkv_bias[2 * D:3 * D][r][None, :]
            ).astype(f16),
        })
    return maps


def kernel(q, k, v, in_w, qkv_bias, out_w, out_b, _trace=False):
    from concourse.bass_utils import run_bass_kernel_spmd

    q = np.asarray(q, dtype=np.float32)
    in_w = np.asarray(in_w, dtype=np.float32)
    qkv_bias = np.asarray(qkv_bias, dtype=np.float32)
    out_w = np.asarray(out_w, dtype=np.float32)
    out_b = np.asarray(out_b, dtype=np.float32)

    nc = _get_nc()
    in_maps = _prep_inputs(q, in_w, qkv_bias)
    for c in range(NCORES):
        r = slice(128 * c, 128 * (c + 1))
        in_maps[c]["w2"] = np.ascontiguousarray(out_w[:, r].T).astype(np.float16)

    res = run_bass_kernel_spmd(
        nc, in_maps, core_ids=list(range(NCORES)), trace=_trace,
    )
    total = np.zeros((D, BS), dtype=np.float32)
    for c in range(NCORES):
        total += res.results[c]["partial"].astype(np.float32)
    net = total.T + out_b[None, :]
    out = net.reshape(B, S, D).astype(np.float32)
    if _trace:
        return out, res
    return out



# revision 5
# speedup vs baseline: 1.1330x; 1.1330x over previous
"""Trainium2 Bass kernel for fused self-attention (nn_Attention).

Reference computes (only q is used; k/v inputs are dead):
    qkv = q @ in_w.T + qkv_bias ; qp,kp,vp = split(qkv)
    per head: softmax(qp @ kp.T / sqrt(hd)) @ vp
    net = concat_heads @ out_w.T + out_b

Sharding: tensor-parallel over heads. 16 heads / 8 cores = 2 heads/core.
Each core projects q against its 2-head slice of in_w, runs attention for
its (2 batch x 2 head) pairs, and computes a partial output projection
against its 128 columns of out_w. Host sums the 8 partials.

Cost-model-driven layout (matmul cost ~= out free size per accumulate
step; ACT cost ~= free size + fixed init):
  scores  [t, s] psum tiles [128, 2tt, 2h, 256s] (2 banks) -> one
          [128, 1024] exp per tile (128 exps total, the ACT floor)
  pv      out [s, e]: lhsT = exp slice [t, s128], rhs = V [t, 65]
          (64 dims + ones column -> denominator). N=65 per accumulate
          step: full PE efficiency, 2x cheaper than the [e, s] form.
          4 accumulators [128, 65] packed in ONE psum bank (start=True
          only on the bank's first matmul, stop=True only on the last;
          first write of each region replaces via pending-zero).
  norm    DVE reciprocal of denom col + per-partition tensor_scalar_mul
          (GpSimd) -> attn [s, d] f16
  transp  PE-transpose [s, d] -> [d, s] (f16 psum), GpSimd copy to outT
  proj    lhsT = w2 slice, rhs = outT [d, s] -> partial [o, s]; DVE copy
          to f16 stage, DMA. Last 256 cols (b1 j7) go psum->DRAM f32
          ("tail" param) to cut the end-of-kernel dependency chain.
  qkv     Q/K bias via per-partition tensor_scalar_add on the psum->sbuf
          copy (no PE cost); V produced per t-tile in [t, vdim] layout
          (no PE transposes), V bias via a 1-row ones matmul.

Schedule: attention spine over (b, j-block of 256 tokens, t-pair).
pv runs one slot behind exp; normalize/transpose of block j ride the
first slots of block j+1. QKV projection and output projection are
deadline-scheduled into the spine's PE slack (weave), streaming against
the q-chunk DMA arrivals. Warmup matmuls hold the PE p-state ramp while
the first q chunk loads.
"""

import sys

for p in ("/opt/trn_rl_repo", "/root/.axon_site/_ro/trn_rl_repo"):
    if p not in sys.path:
        sys.path.append(p)

import numpy as np

B, S, D, H = 2, 2048, 1024, 16
BS = B * S  # 4096
HD = 64  # head dim
NCORES = 8
HPC = H // NCORES  # 2 heads per core -> 128 o-dims per core
JB = 8   # 256-token j-blocks per batch
PP = 8   # t-tile pairs per j-block

_COMPILED = {}


def _build():
    import concourse.bass as bass  # noqa: F401
    import concourse.mybir as mybir
    import concourse.tile as tile
    from concourse import bacc
    from concourse.masks import make_identity

    f16 = mybir.dt.float16
    f32 = mybir.dt.float32
    AF = mybir.ActivationFunctionType

    nc = bacc.Bacc("TRN2", target_bir_lowering=False, debug=False,
                   num_devices=NCORES)

    qT_d = nc.declare_dram_parameter("qT", [D, BS], f16, isOutput=False)
    wqk_d = nc.declare_dram_parameter("wqk", [D, 256], f16, isOutput=False)
    wv_d = nc.declare_dram_parameter("wv", [D, 128], f16, isOutput=False)
    w2_d = nc.declare_dram_parameter("w2", [128, D], f16, isOutput=False)
    qkb_d = nc.declare_dram_parameter("qkb", [128, 2], f32, isOutput=False)
    vb_d = nc.declare_dram_parameter("vb", [1, 128], f16, isOutput=False)
    out_d = nc.declare_dram_parameter("partial", [D, BS], f16, isOutput=True)

    with tile.TileContext(nc) as tc:
        with (
            tc.tile_pool(name="persist", bufs=1) as persist,
            tc.tile_pool(name="exp", bufs=4) as exp_pool,
            tc.tile_pool(name="attn", bufs=2) as attn_pool,
            tc.tile_pool(name="recip", bufs=2) as recip_pool,
            tc.tile_pool(name="stage", bufs=4) as stage_pool,
            tc.tile_pool(name="sc", bufs=2, space="PSUM") as sc_pool,
            tc.tile_pool(name="pv", bufs=2, space="PSUM") as pv_pool,
            tc.tile_pool(name="qkps", bufs=1, space="PSUM") as qk_ps,
            tc.tile_pool(name="wvps", bufs=1, space="PSUM") as wv_ps,
        ):
            # ---- resident SBUF tensors ----
            q_sb = persist.tile([128, 8, BS], f16)      # 64KB/part
            wqk_sb = persist.tile([128, 8, 256], f16)
            wv_sb = persist.tile([128, 8, 128], f16)
            w2_sb = persist.tile([128, D], f16)
            qkb_sb = persist.tile([128, 2], f32)
            vb_sb = persist.tile([1, 128], f16)
            ones_sb = persist.tile([1, 128], f16)
            qk_sb = persist.tile([128, 2, BS], f16)     # [qkdim, Q/K, b*s]
            v_sb = persist.tile([128, B, 16, 130], f16)  # [t, b, tile, dims]
            outT_sb = persist.tile([128, B, 2048], f16)  # [d, b, s]
            ident_sb = persist.tile([128, 128], f16)
            warm_sb = persist.tile([1, 8], f32)

            nc.vector.memset(ones_sb[:, :], 1.0)
            # ones columns of v_sb (64: h0 denom, 129: h1 denom) are set
            # once; per-tile V copies never overwrite them
            nc.vector.memset(v_sb[:, :, :, 64:65], 1.0)
            nc.vector.memset(v_sb[:, :, :, 129:130], 1.0)
            make_identity(nc, ident_sb[:, :])
            # force the exp ACT-table load before DMAs occupy the queues
            nc.vector.memset(warm_sb[:, :], 0.0)
            nc.scalar.activation(warm_sb[:, :], warm_sb[:, :], AF.Exp)

            # loads ordered by first use; q chunk 0 split in halves so the
            # first attention block can start earlier
            qT_t = qT_d.rearrange("(n p) m -> p n m", p=128)
            nc.sync.dma_start(wqk_sb[:, :, :],
                              wqk_d.rearrange("(n p) m -> p n m", p=128))
            nc.sync.dma_start(qkb_sb[:, :], qkb_d[:, :])
            nc.sync.dma_start(q_sb[:, :, 0:256], qT_t[:, :, 0:256])
            nc.sync.dma_start(wv_sb[:, :, :],
                              wv_d.rearrange("(n p) m -> p n m", p=128))
            nc.sync.dma_start(vb_sb[:, :], vb_d[:, :])
            nc.sync.dma_start(q_sb[:, :, 256:512], qT_t[:, :, 256:512])
            for scc in range(1, 4):
                nc.sync.dma_start(
                    q_sb[:, :, scc * 512:(scc + 1) * 512],
                    qT_t[:, :, scc * 512:(scc + 1) * 512])
            nc.sync.dma_start(w2_sb[:, :], w2_d[:, :])
            for scc in range(4, 8):
                nc.sync.dma_start(
                    q_sb[:, :, scc * 512:(scc + 1) * 512],
                    qT_t[:, :, scc * 512:(scc + 1) * 512])

            # ---- work-unit emitters (atomic closures) ------------------
            uid = [0]

            def qj_unit(b, j):
                """Q projection for one 256-token j-block -> qk_sb[:,0,..]"""
                def emit():
                    uid[0] += 1
                    s0 = b * 2048 + j * 256
                    ps = qk_ps.tile([128, 256], f32, tag="qk",
                                    name=f"q{uid[0]}")
                    for dk in range(8):
                        nc.tensor.matmul(
                            ps[:, :], wqk_sb[:, dk, 0:128],
                            q_sb[:, dk, s0:s0 + 256],
                            start=(dk == 0), stop=(dk == 7))
                    nc.vector.tensor_scalar_add(
                        qk_sb[:, 0, s0:s0 + 256], ps[:, :], qkb_sb[:, 0:1])
                return emit

            def k_unit(b, pp):
                """K projection for one t-pair (256 tokens) -> qk_sb[:,1,..]"""
                def emit():
                    uid[0] += 1
                    t0 = b * 2048 + pp * 256
                    ps = qk_ps.tile([128, 256], f32, tag="qk",
                                    name=f"k{uid[0]}")
                    for dk in range(8):
                        nc.tensor.matmul(
                            ps[:, :], wqk_sb[:, dk, 128:256],
                            q_sb[:, dk, t0:t0 + 256],
                            start=(dk == 0), stop=(dk == 7))
                    nc.vector.tensor_scalar_add(
                        qk_sb[:, 1, t0:t0 + 256], ps[:, :], qkb_sb[:, 1:2])
                return emit

            def v_unit(b, st):
                """V projection for one t-tile in [t, vdim] layout."""
                def emit():
                    uid[0] += 1
                    t0 = b * 2048 + st * 128
                    ps = wv_ps.tile([128, 128], f32, tag="wv",
                                    name=f"v{uid[0]}")
                    for dk in range(8):
                        nc.tensor.matmul(
                            ps[:, :], q_sb[:, dk, t0:t0 + 128],
                            wv_sb[:, dk, :],
                            start=(dk == 0), stop=False)
                    nc.tensor.matmul(  # += ones.T @ vb  (per-vdim bias)
                        ps[:, :], ones_sb[0:1, :], vb_sb[0:1, :],
                        start=False, stop=True)
                    nc.gpsimd.tensor_copy(v_sb[:, b, st, 0:64], ps[:, 0:64])
                    nc.gpsimd.tensor_copy(v_sb[:, b, st, 65:129],
                                          ps[:, 64:128])
                return emit

            def proj_part(b, ot, off, w):
                """partial[ot*128:, b*2048+off : +w] via stage copy."""
                def emit():
                    uid[0] += 1
                    ps = wv_ps.tile([128, w], f32, tag="wv",
                                    name=f"p{uid[0]}")
                    nc.tensor.matmul(
                        ps[:, :], w2_sb[:, ot * 128:(ot + 1) * 128],
                        outT_sb[:, b, off:off + w], start=True, stop=True)
                    stg = stage_pool.tile([128, w], f16, tag="st",
                                          name=f"s{uid[0]}")
                    nc.vector.tensor_copy(stg[:, :], ps[:, :])
                    nc.sync.dma_start(
                        out_d[ot * 128:(ot + 1) * 128,
                              b * 2048 + off:b * 2048 + off + w],
                        stg[:, :])
                return emit

            # ---- spine machinery ---------------------------------------
            pv_state = {}

            def emit_pv(e, b, j, p):
                st = pv_state[(b, j)]
                for sub in range(2):
                    step = 2 * p + sub
                    for k in range(2):
                        for h in range(2):
                            nc.tensor.matmul(
                                st["tile"][:, 2 * k + h, :],
                                e[:, sub, h, k * 128:(k + 1) * 128],
                                v_sb[:, b, 2 * p + sub, 65 * h:65 * h + 65],
                                start=(step == 0 and k == 0 and h == 0),
                                stop=(step == 15 and k == 1 and h == 1))

            def norm_block(b, j):
                def emit():
                    pvt = pv_state[(b, j)]["tile"]
                    rc = recip_pool.tile([128, 4, 1], f32, tag="rc",
                                         name=f"rc{b}_{j}")
                    nc.vector.reciprocal(rc[:, :, :], pvt[:, :, 64:65])
                    at = attn_pool.tile([128, 2, 128], f16, tag="at",
                                        name=f"at{b}_{j}")
                    pv_state[(b, j)]["attn"] = at
                    for k in range(2):
                        for h in range(2):
                            nc.gpsimd.tensor_scalar_mul(
                                at[:, k, 64 * h:64 * h + 64],
                                pvt[:, 2 * k + h, 0:64],
                                rc[:, 2 * k + h, 0:1])
                return emit

            def transp_block(b, j):
                def emit():
                    at = pv_state[(b, j)]["attn"]
                    for k in range(2):
                        uid[0] += 1
                        tr = wv_ps.tile([128, 128], f16, tag="wv",
                                        name=f"tr{uid[0]}")
                        nc.tensor.transpose(tr[:, :], at[:, k, :],
                                            ident_sb[:, :])
                        nc.gpsimd.tensor_copy(
                            outT_sb[:, b, j * 256 + k * 128:
                                    j * 256 + (k + 1) * 128], tr[:, :])
                return emit

            # ---- weave schedule ---------------------------------------
            sched = {}

            def at(b, j, p, *units):
                sched.setdefault((b, j, p), []).extend(units)

            # b0 j0: stream b0's K and V against q-chunk arrivals
            at(0, 0, 0, k_unit(0, 1), v_unit(0, 2), v_unit(0, 3))
            at(0, 0, 1, k_unit(0, 2), k_unit(0, 3), v_unit(0, 4))
            at(0, 0, 2, qj_unit(0, 1), v_unit(0, 5), v_unit(0, 6))
            at(0, 0, 3, k_unit(0, 4), k_unit(0, 5), v_unit(0, 7))
            at(0, 0, 4, v_unit(0, 8), v_unit(0, 9), v_unit(0, 10))
            at(0, 0, 5, k_unit(0, 6), k_unit(0, 7), v_unit(0, 11))
            at(0, 0, 6, v_unit(0, 12), v_unit(0, 13))
            at(0, 0, 7, v_unit(0, 14), v_unit(0, 15))
            # b0 j1..j7: rest of Q(b0), all QKV(b1), proj(b0, sc0..2)
            at(0, 1, 0, qj_unit(0, 2))
            at(0, 1, 1, v_unit(1, 0))
            at(0, 1, 2, v_unit(1, 1))
            at(0, 1, 3, k_unit(1, 0))
            at(0, 1, 4, k_unit(1, 1))
            at(0, 1, 5, v_unit(1, 2))
            at(0, 1, 6, v_unit(1, 3), qj_unit(0, 3))
            at(0, 1, 7, v_unit(1, 4))
            at(0, 2, 0, k_unit(1, 2))
            at(0, 2, 1, v_unit(1, 5))
            at(0, 2, 2, v_unit(1, 6))
            at(0, 2, 3, k_unit(1, 3))
            at(0, 2, 4, v_unit(1, 7))
            at(0, 2, 5, qj_unit(0, 4))
            at(0, 2, 6, k_unit(1, 4))
            at(0, 2, 7, v_unit(1, 8))
            at(0, 3, 0, proj_part(0, 0, 0, 512))
            at(0, 3, 1, proj_part(0, 1, 0, 512))
            at(0, 3, 2, v_unit(1, 9))
            at(0, 3, 3, proj_part(0, 2, 0, 512))
            at(0, 3, 4, proj_part(0, 3, 0, 512))
            at(0, 3, 5, k_unit(1, 5))
            at(0, 3, 6, proj_part(0, 4, 0, 512))
            at(0, 3, 7, v_unit(1, 10))
            at(0, 4, 0, proj_part(0, 5, 0, 512))
            at(0, 4, 1, proj_part(0, 6, 0, 512))
            at(0, 4, 2, k_unit(1, 6))
            at(0, 4, 3, proj_part(0, 7, 0, 512))
            at(0, 4, 4, v_unit(1, 11))
            at(0, 4, 5, qj_unit(0, 5))
            at(0, 4, 6, k_unit(1, 7))
            at(0, 4, 7, v_unit(1, 12))
            at(0, 5, 0, v_unit(1, 13))
            at(0, 5, 1, v_unit(1, 14))
            at(0, 5, 2, v_unit(1, 15))
            at(0, 5, 3, qj_unit(0, 6))
            at(0, 5, 4, qj_unit(1, 0))
            at(0, 5, 5, qj_unit(1, 1))
            at(0, 5, 6, proj_part(0, 0, 512, 512))
            at(0, 5, 7, proj_part(0, 1, 512, 512))
            at(0, 6, 0, qj_unit(0, 7))
            at(0, 6, 1, proj_part(0, 2, 512, 512))
            at(0, 6, 2, proj_part(0, 3, 512, 512))
            at(0, 6, 3, proj_part(0, 4, 512, 512))
            at(0, 6, 4, proj_part(0, 5, 512, 512))
            at(0, 6, 5, proj_part(0, 6, 512, 512))
            at(0, 6, 6, proj_part(0, 7, 512, 512))
            for ot in range(8):
                at(0, 7, ot, proj_part(0, ot, 1024, 512))
            # b1: rest of Q(b1), proj(b0, sc3), proj(b1)
            at(1, 0, 0, qj_unit(1, 2))
            at(1, 0, 3, qj_unit(1, 3))
            for ot in range(8):
                at(1, 1, ot, proj_part(0, ot, 1536, 512))
            at(1, 2, 0, qj_unit(1, 4))
            at(1, 2, 1, qj_unit(1, 5))
            for ot in range(8):
                at(1, 3, ot, proj_part(1, ot, 0, 512))
            at(1, 4, 0, qj_unit(1, 6))
            at(1, 4, 1, qj_unit(1, 7))
            for ot in range(8):
                at(1, 5, ot, proj_part(1, ot, 512, 512))
            for i, ot in enumerate(range(5)):
                at(1, 6, 3 + i, proj_part(1, ot, 1024, 512))
            at(1, 7, 0, proj_part(1, 5, 1024, 512))
            at(1, 7, 1, proj_part(1, 6, 1024, 512))
            at(1, 7, 2, proj_part(1, 7, 1024, 512))
            at(1, 7, 3, proj_part(1, 0, 1536, 256), proj_part(1, 1, 1536, 256))
            at(1, 7, 4, proj_part(1, 2, 1536, 256), proj_part(1, 3, 1536, 256))
            at(1, 7, 5, proj_part(1, 4, 1536, 256), proj_part(1, 5, 1536, 256))
            at(1, 7, 6, proj_part(1, 6, 1536, 256), proj_part(1, 7, 1536, 256))

            deferred = {}  # (b, j, p) -> [callables]

            # ---- phase 1: warmup + minimal pre-work --------------------
            wps = wv_ps.tile([128, 128], f32, tag="wv", name="wps")
            for i in range(96):
                nc.tensor.matmul(wps[:, :], ident_sb[:, :], ident_sb[:, :],
                                 start=True, stop=True)
            qj_unit(0, 0)()
            k_unit(0, 0)()
            v_unit(0, 0)()
            v_unit(0, 1)()

            # ---- attention spine ---------------------------------------
            prev_e = None
            for b in range(B):
                for j in range(JB):
                    pv_state[(b, j)] = {
                        "tile": pv_pool.tile([128, 4, 65], f32, tag="pv",
                                             name=f"pv{b}_{j}")}
                    for p in range(PP):
                        s0 = b * 2048 + j * 256
                        sc = sc_pool.tile([128, 2, 2, 256], f32, tag="sc",
                                          name=f"sc{b}_{j}_{p}")
                        for sub in range(2):
                            t0 = b * 2048 + (2 * p + sub) * 128
                            for h in range(2):
                                lo = 64 * h
                                nc.tensor.matmul(
                                    sc[:, sub, h, :],
                                    qk_sb[lo:lo + 64, 1, t0:t0 + 128],
                                    qk_sb[lo:lo + 64, 0, s0:s0 + 256],
                                    start=True, stop=True)
                        for u in deferred.pop((b, j, p), ()):
                            u()
                        for u in sched.pop((b, j, p), ()):
                            u()
                        if prev_e is not None:
                            pe, pb, pj, pp = prev_e
                            emit_pv(pe, pb, pj, pp)
                        e = exp_pool.tile([128, 2, 2, 256], f16, tag="e",
                                          name=f"e{b}_{j}_{p}")
                        nc.scalar.activation(e[:, :, :, :], sc[:, :, :, :],
                                             AF.Exp, scale=0.125)
                        prev_e = (e, b, j, p)
                    # norm/transpose of j ride block j+1's early slots
                    if (b, j) != (1, 7):
                        nb, nj = (b, j + 1) if j < 7 else (b + 1, 0)
                        deferred.setdefault((nb, nj, 1), []).append(
                            norm_block(b, j))
                        deferred.setdefault((nb, nj, 2), []).append(
                            transp_block(b, j))

            assert not sched, f"unconsumed weave slots: {list(sched)}"

            # ---- tail: last block drains, final 256 cols via f32 psum --
            pe, pb, pj, pp = prev_e
            emit_pv(pe, pb, pj, pp)
            norm_block(1, 7)()
            transp_block(1, 7)()
            out_t = out_d.rearrange("(a p) m -> p a m", p=128)
            for half in range(2):
                tps = sc_pool.tile([128, 2, 2, 256], f32, tag="sc",
                                   name=f"tail{half}")
                for i in range(4):
                    ot = half * 4 + i
                    nc.tensor.matmul(
                        tps[:, i // 2, i % 2, :],
                        w2_sb[:, ot * 128:(ot + 1) * 128],
                        outT_sb[:, 1, 1792:2048], start=True, stop=True)
                stg = stage_pool.tile([128, 4, 256], f16, tag="st",
                                      name=f"tstg{half}")
                nc.vector.tensor_copy(stg[:, :, :], tps[:, :, :, :])
                nc.sync.dma_start(
                    out_t[:, half * 4:(half + 1) * 4, 3840:4096],
                    stg[:, :, :])
    nc.compile()
    return nc


def _get_nc():
    if "nc" not in _COMPILED:
        _COMPILED["nc"] = _build()
    return _COMPILED["nc"]


def _prep_inputs(q, in_w, qkv_bias):
    f16 = np.float16
    qT = np.ascontiguousarray(q.transpose(2, 0, 1).reshape(D, BS)).astype(f16)
    maps = []
    for c in range(NCORES):
        r = slice(128 * c, 128 * (c + 1))
        wq, wk, wv = in_w[0:D][r], in_w[D:2 * D][r], in_w[2 * D:3 * D][r]
        maps.append({
            "qT": qT,
            "wqk": np.ascontiguousarray(np.concatenate([wq, wk], 0).T).astype(f16),
            "wv": np.ascontiguousarray(wv.T).astype(f16),
            "w2": None,  # filled with out_w slice
            "qkb": np.ascontiguousarray(
                np.stack([qkv_bias[0:D][r], qkv_bias[D:2 * D][r]], axis=1)
            ).astype(np.float32),
            "vb": np.ascontiguousarray(
                qkv_bias[2 * D:3 * D][r][None, :]).astype(f16),
        })
    return maps


def kernel(q, k, v, in_w, qkv_bias, out_w, out_b, _trace=False):
    from concourse.bass_utils import run_bass_kernel_spmd

    q = np.asarray(q, dtype=np.float32)
    in_w = np.asarray(in_w, dtype=np.float32)
    qkv_bias = np.asarray(qkv_bias, dtype=np.float32)
    out_w = np.asarray(out_w, dtype=np.float32)
    out_b = np.asarray(out_b, dtype=np.float32)

    nc = _get_nc()
    in_maps = _prep_inputs(q, in_w, qkv_bias)
    for c in range(NCORES):
        r = slice(128 * c, 128 * (c + 1))
        in_maps[c]["w2"] = np.ascontiguousarray(out_w[:, r].T).astype(np.float16)

    res = run_bass_kernel_spmd(
        nc, in_maps, core_ids=list(range(NCORES)), trace=_trace,
    )
    total = np.zeros((D, BS), dtype=np.float32)
    for c in range(NCORES):
        total += res.results[c]["partial"].astype(np.float32)
    net = total.T + out_b[None, :]
    out = net.reshape(B, S, D).astype(np.float32)
    if _trace:
        return out, res
    return out


# revision 10
# speedup vs baseline: 1.1749x; 1.0370x over previous
"""Trainium2 Bass kernel for fused self-attention (nn_Attention).

Reference computes (only q is used; k/v inputs are dead):
    qkv = q @ in_w.T + qkv_bias ; qp,kp,vp = split(qkv)
    per head: softmax(qp @ kp.T / sqrt(hd)) @ vp
    net = concat_heads @ out_w.T + out_b

Sharding: tensor-parallel over heads. 16 heads / 8 cores = 2 heads/core.
Each core projects q against its 2-head slice of in_w, runs attention for
its (2 batch x 2 head) pairs, and computes a partial output projection
against its 128 columns of out_w. Host sums the 8 partials.

Cost-model-driven layout (matmul cost ~= out free size per accumulate
step; ACT cost ~= free size + fixed init):
  scores  [t, s] psum tiles [128, 2tt, 2h, 256s] (2 banks) -> one
          [128, 1024] exp per tile (128 exps total, the ACT floor)
  pv      out [s, e]: lhsT = exp slice [t, s128], rhs = V [t, 65]
          (64 dims + ones column -> denominator). N=65 per accumulate
          step: full PE efficiency, 2x cheaper than the [e, s] form.
          4 accumulators [128, 65] packed in ONE psum bank (start=True
          only on the bank's first matmul, stop=True only on the last;
          first write of each region replaces via pending-zero).
  norm    DVE reciprocal of denom col + per-partition tensor_scalar_mul
          (GpSimd) -> attn [s, d] f16
  transp  PE-transpose [s, d] -> [d, s] (f16 psum), GpSimd copy to outT
  proj    lhsT = w2 slice, rhs = outT [d, s] -> partial [o, s]; DVE copy
          to f16 stage, DMA. Last 256 cols (b1 j7) go psum->DRAM f32
          ("tail" param) to cut the end-of-kernel dependency chain.
  qkv     Q/K bias via per-partition tensor_scalar_add on the psum->sbuf
          copy (no PE cost); V produced per t-tile in [t, vdim] layout
          (no PE transposes), V bias via a 1-row ones matmul.

Schedule: attention spine over (b, j-block of 256 tokens, t-pair).
pv runs one slot behind exp; normalize/transpose of block j ride the
first slots of block j+1. QKV projection and output projection are
deadline-scheduled into the spine's PE slack (weave), streaming against
the q-chunk DMA arrivals. Warmup matmuls hold the PE p-state ramp while
the first q chunk loads.
"""

import sys

for p in ("/opt/trn_rl_repo", "/root/.axon_site/_ro/trn_rl_repo"):
    if p not in sys.path:
        sys.path.append(p)

import numpy as np

B, S, D, H = 2, 2048, 1024, 16
BS = B * S  # 4096
HD = 64  # head dim
NCORES = 8
HPC = H // NCORES  # 2 heads per core -> 128 o-dims per core
JB = 8   # 256-token j-blocks per batch
PP = 8   # t-tile pairs per j-block

_COMPILED = {}


def _build():
    import concourse.bass as bass  # noqa: F401
    import concourse.mybir as mybir
    import concourse.tile as tile
    from concourse import bacc
    from concourse.masks import make_identity

    f16 = mybir.dt.float16
    f32 = mybir.dt.float32
    AF = mybir.ActivationFunctionType

    nc = bacc.Bacc("TRN2", target_bir_lowering=False, debug=False,
                   num_devices=NCORES)

    qT_d = nc.declare_dram_parameter("qT", [D, BS], f16, isOutput=False)
    wqk_d = nc.declare_dram_parameter("wqk", [D, 256], f16, isOutput=False)
    wv_d = nc.declare_dram_parameter("wv", [D, 128], f16, isOutput=False)
    w2_d = nc.declare_dram_parameter("w2", [128, D], f16, isOutput=False)
    qkb_d = nc.declare_dram_parameter("qkb", [128, 2], f32, isOutput=False)
    vb_d = nc.declare_dram_parameter("vb", [1, 128], f16, isOutput=False)
    out_d = nc.declare_dram_parameter("partial", [D, BS], f16, isOutput=True)

    with tile.TileContext(nc) as tc:
        with (
            tc.tile_pool(name="persist", bufs=1) as persist,
            tc.tile_pool(name="exp", bufs=4) as exp_pool,
            tc.tile_pool(name="attn", bufs=2) as attn_pool,
            tc.tile_pool(name="recip", bufs=2) as recip_pool,
            tc.tile_pool(name="stage", bufs=4) as stage_pool,
            tc.tile_pool(name="sc", bufs=2, space="PSUM") as sc_pool,
            tc.tile_pool(name="pv", bufs=2, space="PSUM") as pv_pool,
            tc.tile_pool(name="qkps", bufs=1, space="PSUM") as qk_ps,
            tc.tile_pool(name="wvps", bufs=1, space="PSUM") as wv_ps,
        ):
            # ---- resident SBUF tensors ----
            q_sb = persist.tile([128, 8, BS], f16)      # 64KB/part
            wqk_sb = persist.tile([128, 8, 256], f16)
            wv_sb = persist.tile([128, 8, 128], f16)
            w2_sb = persist.tile([128, D], f16)
            qkb_sb = persist.tile([128, 2], f32)
            vb_sb = persist.tile([1, 128], f16)
            ones_sb = persist.tile([1, 128], f16)
            qk_sb = persist.tile([128, 2, BS], f16)     # [qkdim, Q/K, b*s]
            v_sb = persist.tile([128, B, 16, 130], f16)  # [t, b, tile, dims]
            outT_sb = persist.tile([128, B, 2048], f16)  # [d, b, s]
            ident_sb = persist.tile([128, 128], f16)
            warm_sb = persist.tile([1, 8], f32)

            # identity first: the PE warmup stream depends only on it
            make_identity(nc, ident_sb[:, :])
            # force the exp ACT-table load before DMAs occupy the queues
            nc.vector.memset(warm_sb[:, :], 0.0)
            nc.scalar.activation(warm_sb[:, :], warm_sb[:, :], AF.Exp)
            nc.vector.memset(ones_sb[:, :], 1.0)
            # ones columns of v_sb (64: h0 denom, 129: h1 denom) are set
            # once; per-tile V copies never overwrite them
            nc.vector.memset(v_sb[:, :, :, 64:65], 1.0)
            nc.vector.memset(v_sb[:, :, :, 129:130], 1.0)

            # loads ordered by first use; q chunk 0 split in halves so the
            # first attention block can start earlier
            qT_t = qT_d.rearrange("(n p) m -> p n m", p=128)
            nc.sync.dma_start(wqk_sb[:, :, :],
                              wqk_d.rearrange("(n p) m -> p n m", p=128))
            nc.sync.dma_start(qkb_sb[:, :], qkb_d[:, :])
            nc.sync.dma_start(q_sb[:, :, 0:256], qT_t[:, :, 0:256])
            nc.sync.dma_start(wv_sb[:, :, :],
                              wv_d.rearrange("(n p) m -> p n m", p=128))
            nc.sync.dma_start(vb_sb[:, :], vb_d[:, :])
            nc.sync.dma_start(q_sb[:, :, 256:512], qT_t[:, :, 256:512])
            for scc in range(1, 4):
                nc.sync.dma_start(
                    q_sb[:, :, scc * 512:(scc + 1) * 512],
                    qT_t[:, :, scc * 512:(scc + 1) * 512])
            nc.sync.dma_start(w2_sb[:, :], w2_d[:, :])
            for scc in range(4, 8):
                nc.sync.dma_start(
                    q_sb[:, :, scc * 512:(scc + 1) * 512],
                    qT_t[:, :, scc * 512:(scc + 1) * 512])

            # ---- work-unit emitters (atomic closures) ------------------
            uid = [0]

            def qj_unit(b, j):
                """Q projection for one 256-token j-block -> qk_sb[:,0,..]"""
                def emit():
                    uid[0] += 1
                    s0 = b * 2048 + j * 256
                    ps = qk_ps.tile([128, 256], f32, tag="qk",
                                    name=f"q{uid[0]}")
                    for dk in range(8):
                        nc.tensor.matmul(
                            ps[:, :], wqk_sb[:, dk, 0:128],
                            q_sb[:, dk, s0:s0 + 256],
                            start=(dk == 0), stop=(dk == 7))
                    nc.vector.tensor_scalar_add(
                        qk_sb[:, 0, s0:s0 + 256], ps[:, :], qkb_sb[:, 0:1])
                return emit

            def k_unit(b, pp):
                """K projection for one t-pair (256 tokens) -> qk_sb[:,1,..]"""
                def emit():
                    uid[0] += 1
                    t0 = b * 2048 + pp * 256
                    ps = qk_ps.tile([128, 256], f32, tag="qk",
                                    name=f"k{uid[0]}")
                    for dk in range(8):
                        nc.tensor.matmul(
                            ps[:, :], wqk_sb[:, dk, 128:256],
                            q_sb[:, dk, t0:t0 + 256],
                            start=(dk == 0), stop=(dk == 7))
                    nc.vector.tensor_scalar_add(
                        qk_sb[:, 1, t0:t0 + 256], ps[:, :], qkb_sb[:, 1:2])
                return emit

            def v_unit(b, st):
                """V projection for one t-tile in [t, vdim] layout."""
                def emit():
                    uid[0] += 1
                    t0 = b * 2048 + st * 128
                    ps = wv_ps.tile([128, 128], f32, tag="wv",
                                    name=f"v{uid[0]}")
                    for dk in range(8):
                        nc.tensor.matmul(
                            ps[:, :], q_sb[:, dk, t0:t0 + 128],
                            wv_sb[:, dk, :],
                            start=(dk == 0), stop=False)
                    nc.tensor.matmul(  # += ones.T @ vb  (per-vdim bias)
                        ps[:, :], ones_sb[0:1, :], vb_sb[0:1, :],
                        start=False, stop=True)
                    nc.gpsimd.tensor_copy(v_sb[:, b, st, 0:64], ps[:, 0:64])
                    nc.gpsimd.tensor_copy(v_sb[:, b, st, 65:129],
                                          ps[:, 64:128])
                return emit

            def proj_part(b, ot, off, w, eng=None):
                """partial[ot*128:, b*2048+off : +w] via stage copy."""
                def emit():
                    uid[0] += 1
                    ps = wv_ps.tile([128, w], f32, tag="wv",
                                    name=f"p{uid[0]}")
                    nc.tensor.matmul(
                        ps[:, :], w2_sb[:, ot * 128:(ot + 1) * 128],
                        outT_sb[:, b, off:off + w], start=True, stop=True)
                    stg = stage_pool.tile([128, w], f16, tag="st",
                                          name=f"s{uid[0]}")
                    (eng or nc.vector).tensor_copy(stg[:, :], ps[:, :])
                    nc.sync.dma_start(
                        out_d[ot * 128:(ot + 1) * 128,
                              b * 2048 + off:b * 2048 + off + w],
                        stg[:, :])
                return emit

            # ---- spine machinery ---------------------------------------
            pv_state = {}

            def emit_pv(e, b, j, p):
                st = pv_state[(b, j)]
                for sub in range(2):
                    step = 2 * p + sub
                    for k in range(2):
                        for h in range(2):
                            nc.tensor.matmul(
                                st["tile"][:, 2 * k + h, :],
                                e[:, sub, h, k * 128:(k + 1) * 128],
                                v_sb[:, b, 2 * p + sub, 65 * h:65 * h + 65],
                                start=(step == 0 and k == 0 and h == 0),
                                stop=(step == 15 and k == 1 and h == 1))

            def norm_block(b, j):
                def emit():
                    pvt = pv_state[(b, j)]["tile"]
                    rc = recip_pool.tile([128, 4, 1], f32, tag="rc",
                                         name=f"rc{b}_{j}")
                    nc.vector.reciprocal(rc[:, :, :], pvt[:, :, 64:65])
                    at = attn_pool.tile([128, 2, 128], f16, tag="at",
                                        name=f"at{b}_{j}")
                    pv_state[(b, j)]["attn"] = at
                    for k in range(2):
                        for h in range(2):
                            nc.gpsimd.tensor_scalar_mul(
                                at[:, k, 64 * h:64 * h + 64],
                                pvt[:, 2 * k + h, 0:64],
                                rc[:, 2 * k + h, 0:1])
                return emit

            def transp_block(b, j):
                def emit():
                    at = pv_state[(b, j)]["attn"]
                    for k in range(2):
                        uid[0] += 1
                        tr = wv_ps.tile([128, 128], f16, tag="wv",
                                        name=f"tr{uid[0]}")
                        nc.tensor.transpose(tr[:, :], at[:, k, :],
                                            ident_sb[:, :])
                        nc.gpsimd.tensor_copy(
                            outT_sb[:, b, j * 256 + k * 128:
                                    j * 256 + (k + 1) * 128], tr[:, :])
                return emit

            # ---- weave schedule ---------------------------------------
            sched = {}

            def at(b, j, p, *units):
                sched.setdefault((b, j, p), []).extend(units)

            # b0 j0: stream b0's K and V against q-chunk arrivals
            at(0, 0, 0, k_unit(0, 1), v_unit(0, 2), v_unit(0, 3))
            at(0, 0, 1, k_unit(0, 2), k_unit(0, 3), v_unit(0, 4))
            at(0, 0, 2, qj_unit(0, 1), v_unit(0, 5), v_unit(0, 6))
            at(0, 0, 3, k_unit(0, 4), k_unit(0, 5), v_unit(0, 7))
            at(0, 0, 4, v_unit(0, 8), v_unit(0, 9), v_unit(0, 10))
            at(0, 0, 5, k_unit(0, 6), k_unit(0, 7), v_unit(0, 11))
            at(0, 0, 6, v_unit(0, 12), v_unit(0, 13))
            at(0, 0, 7, v_unit(0, 14), v_unit(0, 15))
            # b0 j1..j7: rest of Q(b0), all QKV(b1), proj(b0, sc0..2)
            at(0, 1, 0, qj_unit(0, 2))
            at(0, 1, 1, v_unit(1, 0))
            at(0, 1, 2, v_unit(1, 1))
            at(0, 1, 3, k_unit(1, 0))
            at(0, 1, 4, k_unit(1, 1))
            at(0, 1, 5, v_unit(1, 2))
            at(0, 1, 6, v_unit(1, 3), qj_unit(0, 3))
            at(0, 1, 7, v_unit(1, 4))
            at(0, 2, 0, k_unit(1, 2))
            at(0, 2, 1, v_unit(1, 5))
            at(0, 2, 2, v_unit(1, 6))
            at(0, 2, 3, k_unit(1, 3))
            at(0, 2, 4, v_unit(1, 7))
            at(0, 2, 5, qj_unit(0, 4))
            at(0, 2, 6, k_unit(1, 4))
            at(0, 2, 7, v_unit(1, 8))
            at(0, 3, 0, proj_part(0, 0, 0, 512))
            at(0, 3, 1, proj_part(0, 1, 0, 512))
            at(0, 3, 2, v_unit(1, 9))
            at(0, 3, 3, proj_part(0, 2, 0, 512))
            at(0, 3, 4, proj_part(0, 3, 0, 512))
            at(0, 3, 5, k_unit(1, 5))
            at(0, 3, 6, proj_part(0, 4, 0, 512))
            at(0, 3, 7, v_unit(1, 10))
            at(0, 4, 0, proj_part(0, 5, 0, 512))
            at(0, 4, 1, proj_part(0, 6, 0, 512))
            at(0, 4, 2, k_unit(1, 6))
            at(0, 4, 3, proj_part(0, 7, 0, 512))
            at(0, 4, 4, v_unit(1, 11))
            at(0, 4, 5, qj_unit(0, 5))
            at(0, 4, 6, k_unit(1, 7))
            at(0, 4, 7, v_unit(1, 12))
            at(0, 5, 0, v_unit(1, 13))
            at(0, 5, 1, v_unit(1, 14))
            at(0, 5, 2, v_unit(1, 15))
            at(0, 5, 3, qj_unit(0, 6))
            at(0, 5, 4, qj_unit(1, 0))
            at(0, 5, 5, qj_unit(1, 1))
            at(0, 5, 6, proj_part(0, 0, 512, 512))
            at(0, 5, 7, proj_part(0, 1, 512, 512))
            at(0, 6, 0, qj_unit(0, 7))
            at(0, 6, 1, proj_part(0, 2, 512, 512))
            at(0, 6, 2, proj_part(0, 3, 512, 512))
            at(0, 6, 3, proj_part(0, 4, 512, 512))
            at(0, 6, 4, proj_part(0, 5, 512, 512))
            at(0, 6, 5, proj_part(0, 6, 512, 512))
            at(0, 6, 6, proj_part(0, 7, 512, 512))
            for ot in range(8):
                at(0, 7, ot, proj_part(0, ot, 1024, 512))
            # b1: rest of Q(b1), proj(b0, sc3), proj(b1)
            at(1, 0, 0, qj_unit(1, 2))
            at(1, 0, 3, qj_unit(1, 3))
            for ot in range(8):
                at(1, 1, ot, proj_part(0, ot, 1536, 512))
            at(1, 2, 0, qj_unit(1, 4))
            at(1, 2, 1, qj_unit(1, 5))
            for ot in range(8):
                at(1, 3, ot, proj_part(1, ot, 0, 512))
            at(1, 4, 0, qj_unit(1, 6))
            at(1, 4, 1, qj_unit(1, 7))
            for ot in range(8):
                at(1, 5, ot, proj_part(1, ot, 512, 512))
            # late parts alternate DVE/Pool copies so the tail finds both
            # element engines drained
            gp = nc.gpsimd
            for i, ot in enumerate(range(5)):
                at(1, 6, 3 + i, proj_part(1, ot, 1024, 512,
                                          gp if ot % 2 else None))
            at(1, 7, 0, proj_part(1, 5, 1024, 512, gp))
            at(1, 7, 1, proj_part(1, 6, 1024, 512))
            at(1, 7, 2, proj_part(1, 7, 1024, 512, gp))
            at(1, 7, 3, proj_part(1, 0, 1536, 256),
               proj_part(1, 1, 1536, 256, gp))
            at(1, 7, 4, proj_part(1, 2, 1536, 256),
               proj_part(1, 3, 1536, 256, gp))
            at(1, 7, 5, proj_part(1, 4, 1536, 256),
               proj_part(1, 5, 1536, 256, gp))
            at(1, 7, 6, proj_part(1, 6, 1536, 256),
               proj_part(1, 7, 1536, 256, gp))

            deferred = {}  # (b, j, p) -> [callables]

            # ---- phase 1: warmup + minimal pre-work --------------------
            wps = wv_ps.tile([128, 128], f32, tag="wv", name="wps")
            for i in range(52):
                nc.tensor.matmul(wps[:, :], ident_sb[:, :], ident_sb[:, :],
                                 start=True, stop=True)
            qj_unit(0, 0)()
            k_unit(0, 0)()
            v_unit(0, 0)()
            v_unit(0, 1)()

            # ---- attention spine ---------------------------------------
            # pv runs TWO slots behind exp so a block's last pv never
            # catches up with its exp at the j boundary
            e_queue = []
            for b in range(B):
                for j in range(JB):
                    pv_state[(b, j)] = {
                        "tile": pv_pool.tile([128, 4, 65], f32, tag="pv",
                                             name=f"pv{b}_{j}")}
                    for p in range(PP):
                        s0 = b * 2048 + j * 256
                        sc = sc_pool.tile([128, 2, 2, 256], f32, tag="sc",
                                          name=f"sc{b}_{j}_{p}")
                        for sub in range(2):
                            t0 = b * 2048 + (2 * p + sub) * 128
                            for h in range(2):
                                lo = 64 * h
                                nc.tensor.matmul(
                                    sc[:, sub, h, :],
                                    qk_sb[lo:lo + 64, 1, t0:t0 + 128],
                                    qk_sb[lo:lo + 64, 0, s0:s0 + 256],
                                    start=True, stop=True)
                        for u in deferred.pop((b, j, p), ()):
                            u()
                        for u in sched.pop((b, j, p), ()):
                            u()
                        if len(e_queue) >= 2:
                            emit_pv(*e_queue.pop(0))
                        e = exp_pool.tile([128, 2, 2, 256], f16, tag="e",
                                          name=f"e{b}_{j}_{p}")
                        nc.scalar.activation(e[:, :, :, :], sc[:, :, :, :],
                                             AF.Exp, scale=0.125)
                        e_queue.append((e, b, j, p))
                    # norm/transpose of j ride block j+1's slots 2/3
                    # (pv(j, p7) lands at slot 1 via the lag-2 queue)
                    if (b, j) != (1, 7):
                        nb, nj = (b, j + 1) if j < 7 else (b + 1, 0)
                        deferred.setdefault((nb, nj, 2), []).append(
                            norm_block(b, j))
                        deferred.setdefault((nb, nj, 3), []).append(
                            transp_block(b, j))

            assert not sched, f"unconsumed weave slots: {list(sched)}"

            # ---- tail: drain the lag queue, norm/transpose j7, project
            # the final 256 cols with DVE/Pool split stage copies ----
            for item in e_queue:
                emit_pv(*item)
            norm_block(1, 7)()
            transp_block(1, 7)()
            out_t = out_d.rearrange("(a p) m -> p a m", p=128)
            for half in range(2):
                tps = sc_pool.tile([128, 2, 2, 256], f32, tag="sc",
                                   name=f"tail{half}")
                for i in range(4):
                    ot = half * 4 + i
                    nc.tensor.matmul(
                        tps[:, i // 2, i % 2, :],
                        w2_sb[:, ot * 128:(ot + 1) * 128],
                        outT_sb[:, 1, 1792:2048], start=True, stop=True)
                for pair in range(2):
                    stg = stage_pool.tile([128, 2, 256], f16, tag="st",
                                          name=f"tstg{half}_{pair}")
                    eng = nc.vector if pair == 0 else nc.gpsimd
                    eng.tensor_copy(stg[:, :, :], tps[:, pair, :, :])
                    nc.sync.dma_start(
                        out_t[:, half * 4 + pair * 2:half * 4 + pair * 2 + 2,
                              3840:4096],
                        stg[:, :, :])
    nc.compile()
    return nc


def _get_nc():
    if "nc" not in _COMPILED:
        _COMPILED["nc"] = _build()
    return _COMPILED["nc"]


def _prep_inputs(q, in_w, qkv_bias):
    f16 = np.float16
    qT = np.ascontiguousarray(q.transpose(2, 0, 1).reshape(D, BS)).astype(f16)
    maps = []
    for c in range(NCORES):
        r = slice(128 * c, 128 * (c + 1))
        wq, wk, wv = in_w[0:D][r], in_w[D:2 * D][r], in_w[2 * D:3 * D][r]
        maps.append({
            "qT": qT,
            "wqk": np.ascontiguousarray(np.concatenate([wq, wk], 0).T).astype(f16),
            "wv": np.ascontiguousarray(wv.T).astype(f16),
            "w2": None,  # filled with out_w slice
            "qkb": np.ascontiguousarray(
                np.stack([qkv_bias[0:D][r], qkv_bias[D:2 * D][r]], axis=1)
            ).astype(np.float32),
            "vb": np.ascontiguousarray(
                qkv_bias[2 * D:3 * D][r][None, :]).astype(f16),
        })
    return maps


def kernel(q, k, v, in_w, qkv_bias, out_w, out_b, _trace=False):
    from concourse.bass_utils import run_bass_kernel_spmd

    q = np.asarray(q, dtype=np.float32)
    in_w = np.asarray(in_w, dtype=np.float32)
    qkv_bias = np.asarray(qkv_bias, dtype=np.float32)
    out_w = np.asarray(out_w, dtype=np.float32)
    out_b = np.asarray(out_b, dtype=np.float32)

    nc = _get_nc()
    in_maps = _prep_inputs(q, in_w, qkv_bias)
    for c in range(NCORES):
        r = slice(128 * c, 128 * (c + 1))
        in_maps[c]["w2"] = np.ascontiguousarray(out_w[:, r].T).astype(np.float16)

    res = run_bass_kernel_spmd(
        nc, in_maps, core_ids=list(range(NCORES)), trace=_trace,
    )
    total = np.zeros((D, BS), dtype=np.float32)
    for c in range(NCORES):
        total += res.results[c]["partial"].astype(np.float32)
    net = total.T + out_b[None, :]
    out = net.reshape(B, S, D).astype(np.float32)
    if _trace:
        return out, res
    return out


# revision 14
# speedup vs baseline: 1.1821x; 1.0061x over previous
"""Trainium2 Bass kernel for fused self-attention (nn_Attention).

Reference computes (only q is used; k/v inputs are dead):
    qkv = q @ in_w.T + qkv_bias ; qp,kp,vp = split(qkv)
    per head: softmax(qp @ kp.T / sqrt(hd)) @ vp
    net = concat_heads @ out_w.T + out_b

Sharding: tensor-parallel over heads. 16 heads / 8 cores = 2 heads/core.
Each core projects q against its 2-head slice of in_w, runs attention for
its (2 batch x 2 head) pairs, and computes a partial output projection
against its 128 columns of out_w. Host sums the 8 partials.

Cost-model-driven layout (matmul cost ~= out free size per accumulate
step; ACT cost ~= free size + fixed init):
  scores  [t, s] psum tiles [128, 2tt, 2h, 256s] (2 banks) -> one
          [128, 1024] exp per tile (128 exps total, the ACT floor)
  pv      out [s, e]: lhsT = exp slice [t, s128], rhs = V [t, 65]
          (64 dims + ones column -> denominator). N=65 per accumulate
          step: full PE efficiency, 2x cheaper than the [e, s] form.
          4 accumulators [128, 65] packed in ONE psum bank (start=True
          only on the bank's first matmul, stop=True only on the last;
          first write of each region replaces via pending-zero).
  norm    DVE reciprocal of denom col + per-partition tensor_scalar_mul
          (GpSimd) -> attn [s, d] f16
  transp  PE-transpose [s, d] -> [d, s] (f16 psum), GpSimd copy to outT
  proj    lhsT = w2 slice, rhs = outT [d, s] -> partial [o, s]; DVE copy
          to f16 stage, DMA. Last 256 cols (b1 j7) go psum->DRAM f32
          ("tail" param) to cut the end-of-kernel dependency chain.
  qkv     Q/K bias via per-partition tensor_scalar_add on the psum->sbuf
          copy (no PE cost); V produced per t-tile in [t, vdim] layout
          (no PE transposes), V bias via a 1-row ones matmul.

Schedule: attention spine over (b, j-block of 256 tokens, t-pair).
pv runs one slot behind exp; normalize/transpose of block j ride the
first slots of block j+1. QKV projection and output projection are
deadline-scheduled into the spine's PE slack (weave), streaming against
the q-chunk DMA arrivals. Warmup matmuls hold the PE p-state ramp while
the first q chunk loads.
"""

import sys

for p in ("/opt/trn_rl_repo", "/root/.axon_site/_ro/trn_rl_repo"):
    if p not in sys.path:
        sys.path.append(p)

import numpy as np

B, S, D, H = 2, 2048, 1024, 16
BS = B * S  # 4096
HD = 64  # head dim
NCORES = 8
HPC = H // NCORES  # 2 heads per core -> 128 o-dims per core
JB = 8   # 256-token j-blocks per batch
PP = 8   # t-tile pairs per j-block

_COMPILED = {}


def _build():
    import concourse.bass as bass  # noqa: F401
    import concourse.mybir as mybir
    import concourse.tile as tile
    from concourse import bacc
    from concourse.masks import make_identity

    f16 = mybir.dt.float16
    f32 = mybir.dt.float32
    AF = mybir.ActivationFunctionType

    nc = bacc.Bacc("TRN2", target_bir_lowering=False, debug=False,
                   num_devices=NCORES)

    qT_d = nc.declare_dram_parameter("qT", [D, BS], f16, isOutput=False)
    wqk_d = nc.declare_dram_parameter("wqk", [D, 256], f16, isOutput=False)
    wv_d = nc.declare_dram_parameter("wv", [D, 128], f16, isOutput=False)
    w2_d = nc.declare_dram_parameter("w2", [128, D], f16, isOutput=False)
    qkb_d = nc.declare_dram_parameter("qkb", [128, 2], f32, isOutput=False)
    vb_d = nc.declare_dram_parameter("vb", [1, 128], f16, isOutput=False)
    out_d = nc.declare_dram_parameter("partial", [D, BS], f16, isOutput=True)

    with tile.TileContext(nc) as tc:
        with (
            tc.tile_pool(name="persist", bufs=1) as persist,
            tc.tile_pool(name="exp", bufs=4) as exp_pool,
            tc.tile_pool(name="attn", bufs=2) as attn_pool,
            tc.tile_pool(name="recip", bufs=2) as recip_pool,
            tc.tile_pool(name="stage", bufs=4) as stage_pool,
            tc.tile_pool(name="sc", bufs=2, space="PSUM") as sc_pool,
            tc.tile_pool(name="pv", bufs=2, space="PSUM") as pv_pool,
            tc.tile_pool(name="qkps", bufs=1, space="PSUM") as qk_ps,
            tc.tile_pool(name="wvps", bufs=1, space="PSUM") as wv_ps,
        ):
            # ---- resident SBUF tensors ----
            q_sb = persist.tile([128, 8, BS], f16)      # 64KB/part
            wqk_sb = persist.tile([128, 8, 256], f16)
            wv_sb = persist.tile([128, 8, 128], f16)
            w2_sb = persist.tile([128, D], f16)
            qkb_sb = persist.tile([128, 2], f32)
            vb_sb = persist.tile([1, 128], f16)
            ones_sb = persist.tile([1, 128], f16)
            qk_sb = persist.tile([128, 2, BS], f16)     # [qkdim, Q/K, b*s]
            v_sb = persist.tile([128, B, 16, 130], f16)  # [t, b, tile, dims]
            outT_sb = persist.tile([128, B, 2048], f16)  # [d, b, s]
            ident_sb = persist.tile([128, 128], f16)
            warm_sb = persist.tile([1, 8], f32)

            # identity first: the PE warmup stream depends only on it
            make_identity(nc, ident_sb[:, :])
            # force the exp ACT-table load before DMAs occupy the queues
            nc.vector.memset(warm_sb[:, :], 0.0)
            nc.scalar.activation(warm_sb[:, :], warm_sb[:, :], AF.Exp)
            nc.vector.memset(ones_sb[:, :], 1.0)
            # ones columns of v_sb (64: h0 denom, 129: h1 denom) are set
            # once; per-tile V copies never overwrite them
            nc.vector.memset(v_sb[:, :, :, 64:65], 1.0)
            nc.vector.memset(v_sb[:, :, :, 129:130], 1.0)

            # loads ordered by first use; q chunk 0 split in halves so the
            # first attention block can start earlier
            qT_t = qT_d.rearrange("(n p) m -> p n m", p=128)
            nc.sync.dma_start(wqk_sb[:, :, :],
                              wqk_d.rearrange("(n p) m -> p n m", p=128))
            nc.sync.dma_start(qkb_sb[:, :], qkb_d[:, :])
            nc.sync.dma_start(q_sb[:, :, 0:256], qT_t[:, :, 0:256])
            nc.sync.dma_start(wv_sb[:, :, :],
                              wv_d.rearrange("(n p) m -> p n m", p=128))
            nc.sync.dma_start(vb_sb[:, :], vb_d[:, :])
            nc.sync.dma_start(q_sb[:, :, 256:512], qT_t[:, :, 256:512])
            for scc in range(1, 4):
                nc.sync.dma_start(
                    q_sb[:, :, scc * 512:(scc + 1) * 512],
                    qT_t[:, :, scc * 512:(scc + 1) * 512])
            nc.sync.dma_start(w2_sb[:, :], w2_d[:, :])
            for scc in range(4, 8):
                nc.sync.dma_start(
                    q_sb[:, :, scc * 512:(scc + 1) * 512],
                    qT_t[:, :, scc * 512:(scc + 1) * 512])

            # ---- work-unit emitters (atomic closures) ------------------
            uid = [0]

            def qj_unit(b, j):
                """Q projection for one 256-token j-block -> qk_sb[:,0,..]"""
                def emit():
                    uid[0] += 1
                    s0 = b * 2048 + j * 256
                    ps = qk_ps.tile([128, 256], f32, tag="qk",
                                    name=f"q{uid[0]}")
                    for dk in range(8):
                        nc.tensor.matmul(
                            ps[:, :], wqk_sb[:, dk, 0:128],
                            q_sb[:, dk, s0:s0 + 256],
                            start=(dk == 0), stop=(dk == 7))
                    nc.vector.tensor_scalar_add(
                        qk_sb[:, 0, s0:s0 + 256], ps[:, :], qkb_sb[:, 0:1])
                return emit

            def k_unit(b, pp):
                """K projection for one t-pair (256 tokens) -> qk_sb[:,1,..]"""
                def emit():
                    uid[0] += 1
                    t0 = b * 2048 + pp * 256
                    ps = qk_ps.tile([128, 256], f32, tag="qk",
                                    name=f"k{uid[0]}")
                    for dk in range(8):
                        nc.tensor.matmul(
                            ps[:, :], wqk_sb[:, dk, 128:256],
                            q_sb[:, dk, t0:t0 + 256],
                            start=(dk == 0), stop=(dk == 7))
                    nc.vector.tensor_scalar_add(
                        qk_sb[:, 1, t0:t0 + 256], ps[:, :], qkb_sb[:, 1:2])
                return emit

            def v_unit(b, st):
                """V projection for one t-tile in [t, vdim] layout."""
                def emit():
                    uid[0] += 1
                    t0 = b * 2048 + st * 128
                    ps = wv_ps.tile([128, 128], f32, tag="wv",
                                    name=f"v{uid[0]}")
                    for dk in range(8):
                        nc.tensor.matmul(
                            ps[:, :], q_sb[:, dk, t0:t0 + 128],
                            wv_sb[:, dk, :],
                            start=(dk == 0), stop=False)
                    nc.tensor.matmul(  # += ones.T @ vb  (per-vdim bias)
                        ps[:, :], ones_sb[0:1, :], vb_sb[0:1, :],
                        start=False, stop=True)
                    nc.gpsimd.tensor_copy(v_sb[:, b, st, 0:64], ps[:, 0:64])
                    nc.gpsimd.tensor_copy(v_sb[:, b, st, 65:129],
                                          ps[:, 64:128])
                return emit

            def proj_part(b, ot, off, w, eng=None):
                """partial[ot*128:, b*2048+off : +w] via stage copy."""
                def emit():
                    uid[0] += 1
                    ps = wv_ps.tile([128, w], f32, tag="wv",
                                    name=f"p{uid[0]}")
                    nc.tensor.matmul(
                        ps[:, :], w2_sb[:, ot * 128:(ot + 1) * 128],
                        outT_sb[:, b, off:off + w], start=True, stop=True)
                    stg = stage_pool.tile([128, w], f16, tag="st",
                                          name=f"s{uid[0]}")
                    (eng or nc.vector).tensor_copy(stg[:, :], ps[:, :])
                    nc.sync.dma_start(
                        out_d[ot * 128:(ot + 1) * 128,
                              b * 2048 + off:b * 2048 + off + w],
                        stg[:, :])
                return emit

            # ---- spine machinery ---------------------------------------
            pv_state = {}

            def emit_pv(e, b, j, p):
                st = pv_state[(b, j)]
                for sub in range(2):
                    step = 2 * p + sub
                    for k in range(2):
                        for h in range(2):
                            nc.tensor.matmul(
                                st["tile"][:, 2 * k + h, :],
                                e[:, sub, h, k * 128:(k + 1) * 128],
                                v_sb[:, b, 2 * p + sub, 65 * h:65 * h + 65],
                                start=(step == 0 and k == 0 and h == 0),
                                stop=(step == 15 and k == 1 and h == 1))

            def norm_block(b, j, split=False):
                def emit():
                    pvt = pv_state[(b, j)]["tile"]
                    rc = recip_pool.tile([128, 4, 1], f32, tag="rc",
                                         name=f"rc{b}_{j}")
                    nc.vector.reciprocal(rc[:, :, :], pvt[:, :, 64:65])
                    at = attn_pool.tile([128, 2, 128], f16, tag="at",
                                        name=f"at{b}_{j}")
                    pv_state[(b, j)]["attn"] = at
                    for k in range(2):
                        for h in range(2):
                            eng = nc.vector if (split and h == 0) else nc.gpsimd
                            eng.tensor_scalar_mul(
                                at[:, k, 64 * h:64 * h + 64],
                                pvt[:, 2 * k + h, 0:64],
                                rc[:, 2 * k + h, 0:1])
                return emit

            def transp_block(b, j, split=False):
                def emit():
                    at = pv_state[(b, j)]["attn"]
                    for k in range(2):
                        uid[0] += 1
                        tr = wv_ps.tile([128, 128], f16, tag="wv",
                                        name=f"tr{uid[0]}")
                        nc.tensor.transpose(tr[:, :], at[:, k, :],
                                            ident_sb[:, :])
                        eng = nc.vector if (split and k == 0) else nc.gpsimd
                        eng.tensor_copy(
                            outT_sb[:, b, j * 256 + k * 128:
                                    j * 256 + (k + 1) * 128], tr[:, :])
                return emit

            # ---- weave schedule ---------------------------------------
            sched = {}

            def at(b, j, p, *units):
                sched.setdefault((b, j, p), []).extend(units)

            # b0 j0: stream b0's K and V against q-chunk arrivals
            at(0, 0, 0, k_unit(0, 1), v_unit(0, 2), v_unit(0, 3))
            at(0, 0, 1, k_unit(0, 2), k_unit(0, 3), v_unit(0, 4))
            at(0, 0, 2, qj_unit(0, 1), v_unit(0, 5), v_unit(0, 6))
            at(0, 0, 3, k_unit(0, 4), k_unit(0, 5), v_unit(0, 7))
            at(0, 0, 4, v_unit(0, 8), v_unit(0, 9), v_unit(0, 10))
            at(0, 0, 5, k_unit(0, 6), k_unit(0, 7), v_unit(0, 11))
            at(0, 0, 6, v_unit(0, 12), v_unit(0, 13), qj_unit(0, 2))
            at(0, 0, 7, v_unit(0, 14), v_unit(0, 15))
            # b0 j1..j7: rest of Q(b0), all QKV(b1) (placed after their
            # q-chunk DMA arrivals), proj(b0) filling the chunk-wait slack
            at(0, 1, 0, qj_unit(0, 3))
            at(0, 1, 1, v_unit(1, 0))
            at(0, 1, 2, v_unit(1, 1))
            at(0, 1, 3, k_unit(1, 0))
            at(0, 1, 4, k_unit(1, 1))
            at(0, 1, 5, v_unit(1, 2))
            at(0, 1, 6, v_unit(1, 3))
            at(0, 1, 7, qj_unit(0, 4))
            at(0, 2, 0, k_unit(1, 2))
            at(0, 2, 1, v_unit(1, 4))
            at(0, 2, 2, v_unit(1, 5))
            at(0, 2, 3, proj_part(0, 0, 0, 512))
            at(0, 2, 4, proj_part(0, 1, 0, 512))
            at(0, 2, 5, k_unit(1, 3))
            at(0, 2, 6, v_unit(1, 6))
            at(0, 2, 7, v_unit(1, 7))
            at(0, 3, 0, proj_part(0, 2, 0, 512))
            at(0, 3, 1, proj_part(0, 3, 0, 512))
            at(0, 3, 2, k_unit(1, 4))
            at(0, 3, 3, proj_part(0, 4, 0, 512))
            at(0, 3, 4, proj_part(0, 5, 0, 512))
            at(0, 3, 5, v_unit(1, 8))
            at(0, 3, 6, v_unit(1, 9))
            at(0, 3, 7, proj_part(0, 6, 0, 512))
            at(0, 4, 0, proj_part(0, 7, 0, 512))
            at(0, 4, 1, qj_unit(0, 5))
            at(0, 4, 2, k_unit(1, 5))
            at(0, 4, 3, v_unit(1, 10))
            at(0, 4, 4, v_unit(1, 11))
            at(0, 4, 5, proj_part(0, 0, 512, 512))
            at(0, 4, 6, proj_part(0, 1, 512, 512))
            at(0, 4, 7, k_unit(1, 6))
            at(0, 5, 0, v_unit(1, 12))
            at(0, 5, 1, v_unit(1, 13))
            at(0, 5, 2, proj_part(0, 2, 512, 512))
            at(0, 5, 3, proj_part(0, 3, 512, 512))
            at(0, 5, 4, qj_unit(0, 6))
            at(0, 5, 5, k_unit(1, 7))
            at(0, 5, 6, v_unit(1, 14))
            at(0, 5, 7, v_unit(1, 15))
            at(0, 6, 0, qj_unit(0, 7))
            at(0, 6, 1, proj_part(0, 4, 512, 512))
            at(0, 6, 2, proj_part(0, 5, 512, 512))
            at(0, 6, 3, proj_part(0, 6, 512, 512))
            at(0, 6, 4, proj_part(0, 7, 512, 512))
            at(0, 6, 5, qj_unit(1, 0))
            at(0, 6, 6, qj_unit(1, 1))
            at(0, 7, 0, qj_unit(1, 2))
            at(0, 7, 1, proj_part(0, 0, 1024, 512))
            at(0, 7, 2, proj_part(0, 1, 1024, 512))
            at(0, 7, 3, proj_part(0, 2, 1024, 512))
            at(0, 7, 4, proj_part(0, 3, 1024, 512))
            at(0, 7, 5, proj_part(0, 4, 1024, 512))
            at(0, 7, 6, proj_part(0, 5, 1024, 512))
            at(0, 7, 7, qj_unit(1, 3))
            # b1: rest of Q(b1), proj(b0 tail), all proj(b1)
            at(1, 0, 0, proj_part(0, 6, 1024, 512))
            at(1, 0, 1, proj_part(0, 7, 1024, 512))
            at(1, 0, 4, qj_unit(1, 4))
            at(1, 0, 5, proj_part(0, 0, 1536, 512))
            at(1, 0, 6, proj_part(0, 1, 1536, 512))
            at(1, 0, 7, proj_part(0, 2, 1536, 512))
            at(1, 1, 0, proj_part(0, 3, 1536, 512))
            at(1, 1, 1, proj_part(0, 4, 1536, 512))
            at(1, 1, 2, proj_part(0, 5, 1536, 512))
            at(1, 1, 3, proj_part(0, 6, 1536, 512))
            at(1, 1, 4, proj_part(0, 7, 1536, 512))
            at(1, 1, 5, qj_unit(1, 5))
            at(1, 2, 0, qj_unit(1, 6))
            at(1, 2, 1, qj_unit(1, 7))
            for ot in range(8):
                at(1, 3, ot, proj_part(1, ot, 0, 512))
            for ot in range(8):
                at(1, 5, ot, proj_part(1, ot, 512, 512))
            # late parts alternate DVE/Pool copies so the tail finds both
            # element engines drained
            gp = nc.gpsimd
            for i, ot in enumerate(range(5)):
                at(1, 6, 3 + i, proj_part(1, ot, 1024, 512,
                                          gp if ot % 2 else None))
            at(1, 7, 0, proj_part(1, 5, 1024, 512, gp))
            at(1, 7, 1, proj_part(1, 6, 1024, 512))
            at(1, 7, 2, proj_part(1, 7, 1024, 512, gp))
            at(1, 7, 3, proj_part(1, 0, 1536, 256),
               proj_part(1, 1, 1536, 256, gp))
            at(1, 7, 4, proj_part(1, 2, 1536, 256),
               proj_part(1, 3, 1536, 256, gp))
            at(1, 7, 5, proj_part(1, 4, 1536, 256),
               proj_part(1, 5, 1536, 256, gp))
            at(1, 7, 6, proj_part(1, 6, 1536, 256),
               proj_part(1, 7, 1536, 256, gp))

            deferred = {}  # (b, j, p) -> [callables]

            # ---- phase 1: warmup + minimal pre-work --------------------
            wps = wv_ps.tile([128, 128], f32, tag="wv", name="wps")
            for i in range(22):
                nc.tensor.matmul(wps[:, :], ident_sb[:, :], ident_sb[:, :],
                                 start=True, stop=True)
            qj_unit(0, 0)()
            k_unit(0, 0)()
            v_unit(0, 0)()
            v_unit(0, 1)()

            # ---- attention spine ---------------------------------------
            # pv runs TWO slots behind exp so a block's last pv never
            # catches up with its exp at the j boundary
            e_queue = []
            for b in range(B):
                for j in range(JB):
                    pv_state[(b, j)] = {
                        "tile": pv_pool.tile([128, 4, 65], f32, tag="pv",
                                             name=f"pv{b}_{j}")}
                    for p in range(PP):
                        s0 = b * 2048 + j * 256
                        sc = sc_pool.tile([128, 2, 2, 256], f32, tag="sc",
                                          name=f"sc{b}_{j}_{p}")
                        for sub in range(2):
                            t0 = b * 2048 + (2 * p + sub) * 128
                            for h in range(2):
                                lo = 64 * h
                                nc.tensor.matmul(
                                    sc[:, sub, h, :],
                                    qk_sb[lo:lo + 64, 1, t0:t0 + 128],
                                    qk_sb[lo:lo + 64, 0, s0:s0 + 256],
                                    start=True, stop=True)
                        for u in deferred.pop((b, j, p), ()):
                            u()
                        for u in sched.pop((b, j, p), ()):
                            u()
                        if len(e_queue) >= 2:
                            emit_pv(*e_queue.pop(0))
                        e = exp_pool.tile([128, 2, 2, 256], f16, tag="e",
                                          name=f"e{b}_{j}_{p}")
                        nc.scalar.activation(e[:, :, :, :], sc[:, :, :, :],
                                             AF.Exp, scale=0.125)
                        e_queue.append((e, b, j, p))
                    # norm/transpose of j ride block j+1's slots 2/3
                    # (pv(j, p7) lands at slot 1 via the lag-2 queue)
                    if (b, j) != (1, 7):
                        nb, nj = (b, j + 1) if j < 7 else (b + 1, 0)
                        deferred.setdefault((nb, nj, 2), []).append(
                            norm_block(b, j))
                        deferred.setdefault((nb, nj, 3), []).append(
                            transp_block(b, j))

            assert not sched, f"unconsumed weave slots: {list(sched)}"

            # ---- tail: drain the lag queue, norm/transpose j7, project
            # the final 256 cols with DVE/Pool split stage copies ----
            for item in e_queue:
                emit_pv(*item)
            norm_block(1, 7, split=True)()
            transp_block(1, 7, split=True)()
            out_t = out_d.rearrange("(a p) m -> p a m", p=128)
            for half in range(2):
                tps = sc_pool.tile([128, 2, 2, 256], f32, tag="sc",
                                   name=f"tail{half}")
                for i in range(4):
                    ot = half * 4 + i
                    nc.tensor.matmul(
                        tps[:, i // 2, i % 2, :],
                        w2_sb[:, ot * 128:(ot + 1) * 128],
                        outT_sb[:, 1, 1792:2048], start=True, stop=True)
                stg = stage_pool.tile([128, 4, 256], f16, tag="st",
                                      name=f"tstg{half}")
                nc.vector.tensor_copy(stg[:, 0:2, :], tps[:, 0, :, :])
                nc.gpsimd.tensor_copy(stg[:, 2:4, :], tps[:, 1, :, :])
                nc.sync.dma_start(
                    out_t[:, half * 4:(half + 1) * 4, 3840:4096],
                    stg[:, :, :])
    nc.compile()
    return nc


def _get_nc():
    if "nc" not in _COMPILED:
        _COMPILED["nc"] = _build()
    return _COMPILED["nc"]


def _prep_inputs(q, in_w, qkv_bias):
    f16 = np.float16
    qT = np.ascontiguousarray(q.transpose(2, 0, 1).reshape(D, BS)).astype(f16)
    maps = []
    for c in range(NCORES):
        r = slice(128 * c, 128 * (c + 1))
        wq, wk, wv = in_w[0:D][r], in_w[D:2 * D][r], in_w[2 * D:3 * D][r]
        maps.append({
            "qT": qT,
            "wqk": np.ascontiguousarray(np.concatenate([wq, wk], 0).T).astype(f16),
            "wv": np.ascontiguousarray(wv.T).astype(f16),
            "w2": None,  # filled with out_w slice
            "qkb": np.ascontiguousarray(
                np.stack([qkv_bias[0:D][r], qkv_bias[D:2 * D][r]], axis=1)
            ).astype(np.float32),
            "vb": np.ascontiguousarray(
                qkv_bias[2 * D:3 * D][r][None, :]).astype(f16),
        })
    return maps


def kernel(q, k, v, in_w, qkv_bias, out_w, out_b, _trace=False):
    from concourse.bass_utils import run_bass_kernel_spmd

    q = np.asarray(q, dtype=np.float32)
    in_w = np.asarray(in_w, dtype=np.float32)
    qkv_bias = np.asarray(qkv_bias, dtype=np.float32)
    out_w = np.asarray(out_w, dtype=np.float32)
    out_b = np.asarray(out_b, dtype=np.float32)

    nc = _get_nc()
    in_maps = _prep_inputs(q, in_w, qkv_bias)
    for c in range(NCORES):
        r = slice(128 * c, 128 * (c + 1))
        in_maps[c]["w2"] = np.ascontiguousarray(out_w[:, r].T).astype(np.float16)

    res = run_bass_kernel_spmd(
        nc, in_maps, core_ids=list(range(NCORES)), trace=_trace,
    )
    total = np.zeros((D, BS), dtype=np.float32)
    for c in range(NCORES):
        total += res.results[c]["partial"].astype(np.float32)
    net = total.T + out_b[None, :]
    out = net.reshape(B, S, D).astype(np.float32)
    if _trace:
        return out, res
    return out
